# revision 47
# baseline (speedup 1.0000x reference)
"""Trainium2 Bass kernel for 16-head causal attention with relative position
bias (B=4, S=2048, D=1024, H=16, HD=64), distributed over 8 NeuronCores.

Sharding: tensor-parallel over heads — each core owns 2 heads end-to-end
(QKV projection column-sharded, attention, then an on-device AllToAll
re-shards by tokens so each core runs the output projection for a disjoint
1024-token slice). Host only slices weights / concatenates output slices.

Attention is computed in transposed orientation: scores^T [k, q] via
K @ Q^T, so the P^T needed by the P@V matmul is produced directly by the
exp() — no PE transposes of the probability matrix. The softmax row-sum is
obtained by appending a ones-column to V (row 64 of the attention matmul
accumulator), and 1/rowsum is broadcast with a rank-1 matmul and applied
while copying the accumulator out of PSUM. Key-padding enters through the
per-partition bias operand of the Exp activation (k is the partition dim).

Compute dtype: float32r (full-rate fp32 storage, ~1.5e-4 matmul rel err);
logits, exp and all accumulation stay fp32.

Host-side prep folds work into the data layout:
  - x passed transposed ([D, B*S]) to feed matmuls directly as rhs.
  - HD^-0.5 folded into Wq/bq.
  - causal mask folded into rel_bias, TRANSPOSED per head ([k, q]) and
    shipped as bf16 to halve DMA.
Softmax needs no max-subtraction: logits are O(6), exp is safe in fp32.
"""

import numpy as np
import ml_dtypes

import concourse.bass as bass
import concourse.mybir as mybir
from concourse import bacc
from concourse.tile import TileContext
from concourse.masks import make_identity
from concourse.bass_utils import run_bass_kernel_spmd

B, S, D, H = 4, 2048, 1024, 16
HD = D // H                  # 64
NC_ = 8                      # cores
HPC = H // NC_               # 2 heads per core
T = B * S                    # 8192 tokens
TPC = T // NC_               # 1024 tokens per core (out-proj shard)
NEG = -1e9
FP32 = mybir.dt.float32
F32R = mybir.dt.float32r
BF16 = mybir.dt.bfloat16

NKB = S // 512               # 4 k-blocks (and q macro blocks) per sequence
KTILES = S // 128            # 16 k-tiles per sequence
IDENT = mybir.ActivationFunctionType.Identity
EXP = mybir.ActivationFunctionType.Exp


def build_program() -> bass.Bass:
    """Build the (identical-on-every-core) SPMD Bass program."""
    nc = bacc.Bacc(num_devices=NC_)

    # ---- I/O ----
    xT = nc.dram_tensor("xT", [D, T], BF16, kind="ExternalInput")
    wq = nc.dram_tensor("wq", [D, 128], BF16, kind="ExternalInput")
    wk = nc.dram_tensor("wk", [D, 128], BF16, kind="ExternalInput")
    wv = nc.dram_tensor("wv", [D, 128], BF16, kind="ExternalInput")
    bq = nc.dram_tensor("bq", [128], FP32, kind="ExternalInput")
    bk = nc.dram_tensor("bk", [128], FP32, kind="ExternalInput")
    bv = nc.dram_tensor("bv", [128], FP32, kind="ExternalInput")
    # transposed bias: biasT[h, k, q] = rel_bias[h, q, k] + causal[q, k]
    biasT = nc.dram_tensor("biasT", [HPC, S, S], BF16, kind="ExternalInput")
    # key-padding additive column: pc[p, b, kc] = NEG if token kc*128+p padded
    pc = nc.dram_tensor("pc", [128, B, KTILES], FP32, kind="ExternalInput")
    wout = nc.dram_tensor("wout", [D, D], BF16, kind="ExternalInput")
    boutb = nc.dram_tensor("boutb", [128, D], FP32, kind="ExternalInput")
    npad = nc.dram_tensor("npad", [128, TPC // 128], FP32, kind="ExternalInput")
    out = nc.dram_tensor("out", [TPC, D], FP32, kind="ExternalOutput")

    npads = TPC // 128  # 8 token tiles in out-proj

    with TileContext(nc) as tc:
        with tc.tile_pool(name="const", bufs=1) as const:
            # ---- constants ----
            ident_f32 = const.tile([128, 128], FP32, tag="ident_f32")
            make_identity(nc, ident_f32)
            ident = const.tile([128, 128], F32R, tag="ident")
            nc.vector.tensor_copy(out=ident, in_=ident_f32)
            ident_bf = const.tile([128, 128], BF16, tag="ident_bf")
            nc.vector.tensor_copy(out=ident_bf, in_=ident_f32)
            ones_row = const.tile([1, 64], FP32, tag="ones")
            nc.vector.memset(ones_row, 1.0)
            wq_sb = const.tile([128, 8, 128], BF16, tag="wq")
            wk_sb = const.tile([128, 8, 128], BF16, tag="wk")
            wv_sb = const.tile([128, 8, 128], BF16, tag="wv")
            nc.sync.dma_start(wq_sb, wq.rearrange("(fo p) m -> p fo m", p=128))
            nc.sync.dma_start(wk_sb, wk.rearrange("(fo p) m -> p fo m", p=128))
            nc.sync.dma_start(wv_sb, wv.rearrange("(fo p) m -> p fo m", p=128))
            bq_sb = const.tile([128, 1], FP32, tag="bq")
            bk_sb = const.tile([128, 1], FP32, tag="bk")
            bv_sb = const.tile([128, 1], FP32, tag="bv")
            nc.sync.dma_start(bq_sb, bq[:, None])
            nc.sync.dma_start(bk_sb, bk[:, None])
            nc.sync.dma_start(bv_sb, bv[:, None])
            pc_sb = const.tile([128, B, KTILES], FP32, tag="pc")
            nc.sync.dma_start(pc_sb, pc[:])

            # ---- internal DRAM for the AllToAll ----
            with tc.tile_pool(name="dram", bufs=1, space="DRAM") as dpool:
                a2a_in = [dpool.tile([NC_, 130, TPC // 2], BF16,
                                     tag=f"a2a_in{hf}", name=f"a2a_in{hf}")
                          for hf in range(2)]
                a2a_out = [dpool.tile([NC_, 130, TPC // 2], BF16,
                                      tag=f"a2a_out{hf}", name=f"a2a_out{hf}")
                           for hf in range(2)]

                bigctx = tc.tile_pool(name="big", bufs=1)
                big = bigctx.__enter__()
                # persistent per-core intermediates (phases B+C only)
                # QT/KT: [2*HD qdims (h0 0:64, h1 64:128), B*S tokens]
                QT = big.tile([128, T], BF16, tag="QT")
                KT = big.tile([128, T], BF16, tag="KT")
                # V': [128 token-part, 64 token-chunks, 130]:
                #   cols 0:64 head0, 64 ones, 65:129 head1, 129 ones
                V = big.tile([128, T // 128, 130], BF16, tag="V")
                nc.vector.memset(V[:, :, 64:65], 1.0)
                nc.vector.memset(V[:, :, 129:130], 1.0)

                # ================= Phase B: QKV projection =================
                xT_r = xT.rearrange("(fo p) t -> p fo t", p=128)
                with tc.tile_pool(name="qkv", bufs=3) as qkvp, \
                     tc.tile_pool(name="qkv_ps", bufs=2, space="PSUM") as qps:
                    for tb in range(T // 512):
                        xt = qkvp.tile([128, 8, 512], BF16, tag="xt")
                        nc.sync.dma_start(xt, xT_r[:, :, tb * 512:(tb + 1) * 512])
                        psq = qps.tile([128, 512], FP32, tag="psq")
                        psk = qps.tile([128, 512], FP32, tag="psk")
                        psv = qps.tile([128, 512], FP32, tag="psv")
                        for fo in range(8):
                            nc.tensor.matmul(psq, wq_sb[:, fo], xt[:, fo],
                                             start=(fo == 0), stop=(fo == 7))
                        for fo in range(8):
                            nc.tensor.matmul(psk, wk_sb[:, fo], xt[:, fo],
                                             start=(fo == 0), stop=(fo == 7))
                        for fo in range(8):
                            nc.tensor.matmul(psv, wv_sb[:, fo], xt[:, fo],
                                             start=(fo == 0), stop=(fo == 7))
                        sl = slice(tb * 512, (tb + 1) * 512)
                        nc.scalar.activation(QT[:, sl], psq, IDENT, bias=bq_sb)
                        nc.scalar.activation(KT[:, sl], psk, IDENT, bias=bk_sb)
                        # V^T [vdim, tok] -> transpose to V [tok, vdim]
                        vt = qkvp.tile([128, 512], F32R, tag="vt")
                        nc.scalar.activation(vt, psv, IDENT, bias=bv_sb)
                        for t4 in range(4):
                            pst = qps.tile([128, 128], F32R, tag="pst")
                            nc.tensor.transpose(
                                pst, vt[:, t4 * 128:(t4 + 1) * 128], ident)
                            c = tb * 4 + t4
                            nc.scalar.add(V[:, c, 0:64], pst[:, 0:64], 0.0)
                            nc.scalar.add(V[:, c, 65:129], pst[:, 64:128], 0.0)

                # ================= Phase C: attention =================
                with tc.tile_pool(name="att", bufs=6) as att, \
                     tc.tile_pool(name="attb", bufs=2) as attb, \
                     tc.tile_pool(name="pt", bufs=6) as ptp, \
                     tc.tile_pool(name="att_ps", bufs=3, space="PSUM") as aps, \
                     tc.tile_pool(name="av_ps", bufs=1, space="PSUM") as avps:
                    for qmb in (0, 2, 1, 3):
                        nkt = (qmb + 1) * 4  # k-tiles needed (causal)
                        # bias^T tiles for this qmb, both heads, cached
                        # across the 4 batches
                        bias_ts = {}
                        for h in range(HPC):
                            for kc in range(nkt):
                                j = kc - qmb * 4
                                off = max(j, 0) * 128
                                bt = attb.tile([128, 512], BF16,
                                               tag=f"bt{h}_{kc}")
                                nc.sync.dma_start(
                                    bt[:, off:],
                                    biasT[h, kc * 128:(kc + 1) * 128,
                                          qmb * 512 + off:(qmb + 1) * 512])
                                bias_ts[(h, kc)] = bt
                        for b in range(B):
                            avs = [avps.tile([65, 512], FP32, tag=f"av{h}",
                                             name=f"av{h}_{qmb}_{b}")
                                   for h in range(HPC)]
                            qsl = slice(b * S + qmb * 512,
                                        b * S + (qmb + 1) * 512)
                            # one-iteration skew: kc's P@V is emitted
                            # after kc+1's score matmuls so the PE stream
                            # always has independent work between dependent
                            # accumulate steps
                            pts = {}
                            def emit_av(kc):
                                off_a = max(kc - qmb * 4, 0) * 128
                                for h in range(HPC):
                                    vsl = slice(h * 65, h * 65 + 65)
                                    nc.tensor.matmul(
                                        avs[h][:, off_a:] if kc else avs[h],
                                        V[:, b * 16 + kc, vsl],
                                        pts[(h, kc)][:, off_a:] if kc
                                        else pts[(h, kc)],
                                        start=(kc == 0),
                                        stop=(kc == nkt - 1))
                                del pts[(0, kc)], pts[(1, kc)]
                            for kc in range(nkt):
                                # columns [0, off) of this k-tile are fully
                                # causally masked -> skip them
                                j = kc - qmb * 4
                                off = max(j, 0) * 128
                                pss = []
                                for h in range(HPC):
                                    hsl = slice(h * 64, h * 64 + 64)
                                    ps = aps.tile([128, 512], FP32,
                                                  tag=f"s_ps{h}")
                                    # adjacent h0/h1 matmuls pack into PE
                                    # row-groups 0:64 / 64:128
                                    nc.tensor.matmul(
                                        ps[:, off:],
                                        KT[hsl, b * S + kc * 128:
                                           b * S + (kc + 1) * 128],
                                        QT[hsl, b * S + qmb * 512 + off:
                                           b * S + (qmb + 1) * 512],
                                        start=True, stop=True)
                                    pss.append(ps)
                                for h in range(HPC):
                                    s_sb = att.tile([128, 512], FP32,
                                                    tag=f"s{h}")
                                    nc.vector.tensor_add(
                                        out=s_sb[:, off:], in0=pss[h][:, off:],
                                        in1=bias_ts[(h, kc)][:, off:])
                                    pt = ptp.tile([128, 512], BF16,
                                                  tag=f"pt{h}")
                                    nc.scalar.activation(
                                        pt[:, off:], s_sb[:, off:], EXP,
                                        bias=pc_sb[:, b, kc:kc + 1])
                                    pts[(h, kc)] = pt
                                if kc >= 1:
                                    emit_av(kc - 1)
                            emit_av(nkt - 1)
                            g = b * S + qmb * 512
                            half = (g % TPC) // 512
                            for h in range(HPC):
                                av_sb = att.tile([65, 512], BF16,
                                                 tag=f"avsb{h}")
                                nc.vector.tensor_copy(out=av_sb, in_=avs[h])
                                nc.sync.dma_start(
                                    a2a_in[half][g // TPC,
                                                 h * 64:h * 64 + 64, :],
                                    av_sb[0:64, :])
                                nc.sync.dma_start(
                                    a2a_in[half][g // TPC, 128 + h, :],
                                    av_sb[64:65, :])

                bigctx.__exit__(None, None, None)

                # ============== Phase D: AllToAll + out-proj ==============
                with tc.tile_pool(name="proj", bufs=1) as proj, \
                     tc.tile_pool(name="proj_w", bufs=2) as projw, \
                     tc.tile_pool(name="proj_ps", bufs=2, space="PSUM") as pps:
                    wout_sb = proj.tile([128, 8, D], BF16, tag="wout")
                    nc.sync.dma_start(
                        wout_sb, wout.rearrange("(io p) n -> p io n", p=128))
                    boutb_sb = proj.tile([128, D], FP32, tag="boutb")
                    nc.sync.dma_start(boutb_sb, boutb[:])
                    npad_sb = proj.tile([128, npads], FP32, tag="npad")
                    nc.sync.dma_start(npad_sb, npad[:])
                    HT = TPC // 2  # tokens per half
                    for hf in range(2):
                        nc.gpsimd.collective_compute(
                            "AllToAll", mybir.AluOpType.bypass,
                            replica_groups=[list(range(NC_))],
                            ins=[a2a_in[hf][:]], outs=[a2a_out[hf][:]])
                        recv = []
                        recvz = []
                        for i in range(NC_):
                            r = proj.tile([128, HT], BF16, tag=f"recv{hf}_{i}")
                            nc.sync.dma_start(r, a2a_out[hf][i, 0:128, :])
                            recv.append(r)
                            rzs = []
                            for hh in range(2):
                                rzb = proj.tile([1, HT], BF16,
                                                tag=f"recvzb{hf}_{i}_{hh}")
                                nc.sync.dma_start(
                                    rzb, a2a_out[hf][i, 128 + hh:129 + hh, :])
                                rz = proj.tile([1, HT], FP32,
                                               tag=f"recvz{hf}_{i}_{hh}")
                                nc.vector.tensor_copy(out=rz, in_=rzb)
                                rzs.append(rz)
                            recvz.append(rzs)
                        # normalize by 1/Z via reciprocal + rank-1 broadcast
                        for i in range(NC_):
                            rcs = []
                            for hh in range(2):
                                rc_d = projw.tile([1, HT], FP32,
                                                  tag=f"rc_d{hh}")
                                nc.vector.reciprocal_approx_fast(
                                    out=rc_d, in_=recvz[i][hh][:])
                                rcs.append(rc_d)
                            bc_ps = pps.tile([128, 512], FP32, tag="bc_ps")
                            nc.tensor.matmul(
                                bc_ps[0:64, :], ones_row, rcs[0][:],
                                start=True, stop=True)
                            nc.tensor.matmul(
                                bc_ps[64:128, :], ones_row, rcs[1][:],
                                start=True, stop=True, tile_position=(0, 64))
                            nc.vector.tensor_tensor(
                                out=recv[i][:], in0=recv[i][:],
                                in1=bc_ps, op=mybir.AluOpType.mult)
                        for tt4 in range(npads // 2):
                            tt = hf * (npads // 2) + tt4
                            o_sb = projw.tile([128, D], FP32, tag="osb")
                            for nb in range(2):
                                ps = pps.tile([128, 512], FP32, tag="o_ps")
                                for i in range(NC_):
                                    nc.tensor.matmul(
                                        ps,
                                        recv[i][:, tt4 * 128:(tt4 + 1) * 128],
                                        wout_sb[:, i, nb * 512:(nb + 1) * 512],
                                        start=(i == 0), stop=(i == NC_ - 1))
                                nsl = slice(nb * 512, (nb + 1) * 512)
                                nc.vector.tensor_add(
                                    out=o_sb[:, nsl], in0=ps,
                                    in1=boutb_sb[:, nsl])
                            nc.vector.tensor_scalar_mul(
                                o_sb, o_sb, npad_sb[:, tt:tt + 1])
                            nc.sync.dma_start(
                                out[tt * 128:(tt + 1) * 128, :], o_sb)
    nc.finalize()
    return nc


_CACHE: dict = {}


def _prep_inputs(x, Wqkv, bqkv, Wout, bout, causal_mask, rel_bias,
                 key_padding_mask):
    """Host-side shard prep: returns in_maps."""
    f32 = np.float32
    bf16 = ml_dtypes.bfloat16
    x = np.asarray(x, f32)
    Wqkv = np.asarray(Wqkv, f32)
    bqkv = np.asarray(bqkv, f32)
    Wout = np.asarray(Wout, f32)
    bout = np.asarray(bout, f32)
    causal_mask = np.asarray(causal_mask, f32)
    rel_bias = np.asarray(rel_bias, f32)
    kpm = np.asarray(key_padding_mask, bool)

    scale = f32(HD ** -0.5)
    xT = np.ascontiguousarray(x.reshape(T, D).T.astype(bf16))

    # key-padding additive column per k-tile: [128, B, KTILES]
    pcm = np.where(kpm, f32(NEG), f32(0.0)).astype(f32)       # [B, S]
    pcm = np.ascontiguousarray(
        pcm.reshape(B, KTILES, 128).transpose(2, 0, 1))       # [128, B, KT]
    boutb = np.ascontiguousarray(np.broadcast_to(bout[None], (128, D)))
    notpad_flat = (~kpm).reshape(T).astype(f32)

    in_maps = []
    for c in range(NC_):
        co = 128 * c
        wq_c = np.ascontiguousarray((Wqkv[:, co:co + 128] * scale).astype(bf16))
        wk_c = np.ascontiguousarray(Wqkv[:, D + co:D + co + 128].astype(bf16))
        wv_c = np.ascontiguousarray(Wqkv[:, 2 * D + co:2 * D + co + 128].astype(bf16))
        bq_c = np.ascontiguousarray(bqkv[co:co + 128] * scale)
        bk_c = np.ascontiguousarray(bqkv[D + co:D + co + 128])
        bv_c = np.ascontiguousarray(bqkv[2 * D + co:2 * D + co + 128])
        bias_c = rel_bias[HPC * c:HPC * c + HPC] + causal_mask[None]
        biasT_c = np.ascontiguousarray(
            bias_c.transpose(0, 2, 1).astype(bf16))
        np_c = np.ascontiguousarray(
            notpad_flat[c * TPC:(c + 1) * TPC].reshape(TPC // 128, 128).T)
        in_maps.append({
            "xT": xT, "wq": wq_c, "wk": wk_c, "wv": wv_c,
            "bq": bq_c, "bk": bk_c, "bv": bv_c,
            "biasT": biasT_c, "pc": pcm,
            "wout": np.ascontiguousarray(Wout.astype(bf16)),
            "boutb": boutb, "npad": np_c,
        })
    return in_maps


def kernel(**inputs) -> np.ndarray:
    in_maps = _prep_inputs(**inputs)
    if "prog" not in _CACHE:
        _CACHE["prog"] = build_program()
    nc = _CACHE["prog"]
    res = run_bass_kernel_spmd(nc, in_maps, core_ids=list(range(NC_)))
    outs = [res.results[c]["out"] for c in range(NC_)]
    return np.concatenate(outs, axis=0).reshape(B, S, D)
